# revision 61
# baseline (speedup 1.0000x reference)
"""AttentionAugmentedConv2d Trainium2 Bass kernel (v3.8, ~185us HW).

Data-parallel over batch: 8 samples -> 8 NeuronCores, one sample per core.
Self-contained: hardcodes all shapes; builds derived constant inputs on host.

Changes vs the 188us v2 baseline:
  - Pre-phase restructure: heads 0-3 rel+skew+staging complete before the
    first logits (~35us vs ~57us). Weights ride the gpsimd SWDGE so the
    skew bounce never queues behind 4.7MB of HWDGE weight traffic; ALL
    skew write/read triggers live on the SP engine (a 2016-descriptor
    dma_start costs 1-5us of engine time and in v2/v3 head-of-line
    blocked the ACT exp stream or the staging reads).
  - Rel tables: 4 heads per matmul pass via row+col tile_position
    ((32g, 64*(g%2)), K=32 M=64) - 4x fewer rel matmul cycles; each PSUM
    bank evacuated as ONE full 128-row copy (DVE time scales with free
    size only), heads paired per rsb tile.
  - Scale folding: SCALE rides the k epilogue and the host krcb2 tables,
    so the q epilogues are bias-only; q kept in two layouts (q_st y-major
    for logits/tab0, q_nat contiguous for rel tab1 - strided moving
    operands run 2.7x slower on the PE).
  - v->vT transposes via XBAR dma_start_transpose (contiguous scratch +
    DVE copy) instead of PE transpose-mode: PE freed, PSUM traffic gone.
  - vT ones columns + warm scratch via DVE memsets (the v2 broadcast
    DMAs moved 1.2MB at 37GB/s through the critical queues).
  - 12 warm-up matmuls on a zeroed scratch during the x-DMA wait get the
    HAM clock gate to 8/8 before q0 starts.
  - att (and pair-end transposes) emitted BEFORE each iteration's filler
    so the aps PSUM recycle isn't gated behind filler epilogues on DVE.
  - Final 1x1 conv split: W0 @ attn[heads 0-3] + bias accumulates into
    SBUF right after pair 1 completes; the tail keeps only
    W1 @ attn[heads 4-7] + add + store.
  - x DMA split so the st0 conv rows land before the q0 weights.
"""

import sys

sys.path.insert(0, "/opt/trn_rl_repo")

import numpy as np

import concourse.bass as bass
import concourse.tile as tile
from concourse import bacc, mybir
from concourse.bass_types import AP

F32 = mybir.dt.float32
BF16 = mybir.dt.bfloat16

B, CIN, COUT, DK, DV, NH, H, W = 8, 256, 512, 256, 256, 8, 32, 32
DKH = DK // NH  # 32
HW = H * W  # 1024
SCALE = DKH ** -0.5
N_CORES = 8
FILL_PER_U = 8  # conv MMs of filler per logits u-group
LAG = 3

_cached = {}


def build_bass():
    nc = bacc.Bacc("TRN2", target_bir_lowering=False, debug=False,
                   num_devices=N_CORES)

    x_d = nc.dram_tensor("x", [2, 128, 34 * 34], BF16,
                         kind="ExternalInput").ap()
    # conv weights: (cout_tile, ci_half, ci_local, tap, co_local);
    # qkv tiles 0-5 (q0 q1 k0 k1 v0 v1), conv tiles 6-7
    wq_d = nc.dram_tensor("wqkvT", [6, 2, 128, 9, 128], BF16,
                          kind="ExternalInput").ap()
    wc_d = nc.dram_tensor("wconvT", [2, 2, 128, 9, 128], BF16,
                          kind="ExternalInput").ap()
    wattT_d = nc.dram_tensor("wattT", [2, 128, 256], BF16,
                             kind="ExternalInput").ap()
    bcol_d = nc.dram_tensor("bias_cols", [128, 10], F32,
                            kind="ExternalInput").ap()
    krc_d = nc.dram_tensor("krcb2", [2, 128, 128], BF16,
                           kind="ExternalInput").ap()
    ltc_d = nc.dram_tensor("ltc", [96, HW], BF16, kind="ExternalInput").ap()
    rtz_d = nc.dram_tensor("rtz", [32, HW], BF16,
                           kind="ExternalInput").ap()
    out_d = nc.dram_tensor("out", [COUT, HW], F32, kind="ExternalOutput").ap()

    with tile.TileContext(nc) as tc:
        _build(nc, tc, x_d, wq_d, wc_d, wattT_d, bcol_d, krc_d,
               ltc_d, rtz_d, out_d)
    nc.compile()
    return nc


def _build(nc, tc, x_d, wq_d, wc_d, wattT_d, bcol_d, krc_d,
           ltc_d, rtz_d, out_d):
    from contextlib import ExitStack

    Exp = mybir.ActivationFunctionType.Exp
    ADD = mybir.AluOpType.add
    MULT = mybir.AluOpType.mult

    ctx = ExitStack()
    with ctx:
        persist = ctx.enter_context(tc.tile_pool(name="persist", bufs=1))
        wpool = ctx.enter_context(tc.tile_pool(name="wpool", bufs=16))
        xfer = ctx.enter_context(tc.tile_pool(name="xfer", bufs=2))
        e_pool = ctx.enter_context(tc.tile_pool(name="epool", bufs=10))
        rel_pool = ctx.enter_context(tc.tile_pool(name="relsb", bufs=4))
        psum = ctx.enter_context(
            tc.tile_pool(name="psum", bufs=2, space="PSUM"))
        dram_pool = ctx.enter_context(
            tc.tile_pool(name="dramp", bufs=1, space="DRAM"))

        # ---------- persistent SBUF ----------
        xpad = [persist.tile([128, 34 * 34], BF16, tag=f"xpad{i}",
                             name=f"xpad{i}") for i in range(2)]
        q_st = [persist.tile([128, HW], BF16, tag=f"qst{i}", name=f"qst{i}")
                for i in range(2)]
        q_nat = [persist.tile([128, HW], BF16, tag=f"qnat{i}",
                              name=f"qnat{i}") for i in range(2)]
        k_st = [persist.tile([128, HW], BF16, tag=f"kst{i}", name=f"kst{i}")
                for i in range(2)]
        v_st = [persist.tile([128, HW], BF16, tag=f"vst{i}", name=f"vst{i}")
                for i in range(2)]
        vT_t = [persist.tile([128, 576], BF16, tag=f"vT{u}", name=f"vT{u}")
                for u in range(8)]
        wattT_sb = [persist.tile([128, 256], BF16, tag=f"watt{i}",
                                 name=f"watt{i}") for i in range(2)]
        bcol = persist.tile([128, 10], F32, tag="bcol", name="bcol")
        # rel tables: (key_rel*SCALE)^T for head-block g at rows 32g,
        # cols 64*(g%2)
        krc2 = [persist.tile([128, 128], BF16, tag=f"krc2{t}",
                             name=f"krc2{t}") for t in range(2)]
        att_un = [persist.tile([128, HW], F32, tag=f"attun{g}",
                               name=f"attun{g}") for g in range(2)]
        smap = [persist.tile([128, HW], F32, tag=f"smap{g}", name=f"smap{g}")
                for g in range(2)]
        rmap = [persist.tile([128, HW], F32, tag=f"rmap{g}", name=f"rmap{g}")
                for g in range(2)]
        attn = [persist.tile([128, HW], BF16, tag=f"attn{g}", name=f"attn{g}")
                for g in range(2)]
        opart = [persist.tile([128, 512], F32, tag=f"opart{j}",
                              name=f"opart{j}") for j in range(4)]

        # lt ring: [k_h; I_w; I_h; 0] bf16; rt ring: [q_h; A_w; A_h'; 0]
        lt_r = [persist.tile([128, HW], BF16, tag=f"lt{s}", name=f"lt{s}")
                for s in range(4)]
        rt_r = [persist.tile([128, HW], BF16, tag=f"rt{s}", name=f"rt{s}")
                for s in range(4)]
        skw = [[dram_pool.tile([94 * HW], BF16, tag=f"skw{h}_{t}",
                               name=f"skw{h}_{t}") for t in range(2)]
               for h in range(NH)]

        # ---------- startup DMAs ----------
        # sync/scalar HWDGE carry x/weights for q0 plus all the constants;
        # the constants sit in queue positions that drain while the queue
        # would otherwise idle waiting for the (data-gated) skew writes.
        # Bulk weights go via gpsimd SWDGE so the skew bounce never queues
        # behind them.
        # split x so the st0 conv rows (0-17) land before the q0 weights
        nc.sync.dma_start(xpad[0][:, 0:612], x_d[0][:, 0:612])
        nc.scalar.dma_start(xpad[1][:, 0:612], x_d[1][:, 0:612])
        wq0 = []
        for c in range(2):
            wt = wpool.tile([128, 9 * 128], BF16, tag="w", name=f"w_q0{c}")
            eng = nc.sync if c == 0 else nc.scalar
            eng.dma_start(wt[:], wq_d[0][c].rearrange("p t co -> p (t co)"))
            wq0.append(wt)
        nc.sync.dma_start(xpad[0][:, 612:1156], x_d[0][:, 612:1156])
        nc.scalar.dma_start(xpad[1][:, 612:1156], x_d[1][:, 612:1156])
        nc.scalar.dma_start(bcol[:], bcol_d[:])
        # lt/rt ring const rows: slots 0/1 now (first logits), slots 2/3
        # deferred to after the pre-phase skew traffic (needed at pair 1)
        for s in range(2):
            eng = nc.sync if s % 2 == 0 else nc.scalar
            eng.dma_start(lt_r[s][32:128, :], ltc_d[:])
            eng.dma_start(rt_r[s][96:128, :], rtz_d[:])
        # warm-up fodder: the PE idles ~6us waiting for x/weights at
        # startup; dummy matmuls there get HAM to K=8/8 before q0 starts
        warm = persist.tile([128, 512], BF16, tag="warm", name="warm")
        nc.vector.memset(warm[:], 0.0)
        # vT softmax-denominator ones columns: cols 32:64 of head blocks
        # 0-7 (cols 0:32 are fully written by the vt transposes before any
        # att LDW reads them; block 8 is never read)
        for u in range(8):
            nc.vector.memset(
                vT_t[u][:].rearrange("p (h j) -> p h j", h=9)[:, 0:8, 32:64],
                1.0)

        # ---------- conv blocks as resumable filler jobs ----------
        def xwin(cih, r0, nr, dy, dx):
            v = xpad[cih][:].rearrange("p (a b) -> p a b", a=34)
            return v[:, dy + r0: dy + r0 + nr, dx:dx + 32]

        class Block:
            def __init__(self, w_src, epi, name):
                self.w_src, self.epi, self.name = w_src, epi, name
                self.ws = None
                self.st = 0
                self.i = 0
                self.cps = None
                self.done = False

            def prefetch(self):
                if self.ws is None:
                    self.ws = []
                    for c in range(2):
                        wt = wpool.tile([128, 9 * 128], BF16, tag="w",
                                        name=f"w_{self.name}{c}")
                        nc.gpsimd.dma_start(
                            wt[:],
                            self.w_src[c].rearrange("p t co -> p (t co)"))
                        self.ws.append(wt)

            def emit(self, n):
                self.prefetch()
                emitted = 0
                while n > 0 and not self.done:
                    if self.i == 0:
                        self.cps = psum.tile([128, 512], F32, tag="cps",
                                             name=f"c_{self.name}{self.st}")
                    # all c=0 taps first: the c=1 weight/x tiles arrive on
                    # the scalar queue a few us later at startup
                    t, c = self.i % 9, self.i // 9
                    dy, dx = t // 3, t % 3
                    nc.tensor.matmul(
                        self.cps[:], self.ws[c][:, 128 * t:128 * (t + 1)],
                        xwin(c, 16 * self.st, 16, dy, dx),
                        start=(self.i == 0), stop=(self.i == 17))
                    self.i += 1
                    emitted += 1
                    n -= 1
                    if self.i == 18:
                        self.epi(self.st, self.cps)
                        self.i = 0
                        self.st += 1
                        self.done = self.st == 2
                return emitted

        def qkv_epi(cc):
            def epi(st, cps):
                b = bcol[:, cc:cc + 1]
                if cc < 2:
                    qv = q_st[cc][:].rearrange("p (c r) -> p r c", r=32)
                    nc.vector.tensor_scalar(
                        qv[:, 16 * st:16 * (st + 1), :],
                        cps[:].rearrange("p (r c) -> p r c", r=16),
                        b, None, ADD)
                    nc.vector.tensor_scalar(
                        q_nat[cc][:, 512 * st:512 * (st + 1)], cps[:],
                        b, None, ADD)
                elif cc < 4:
                    nc.vector.tensor_scalar(
                        k_st[cc - 2][:, 512 * st:512 * (st + 1)], cps[:],
                        b, SCALE, ADD, MULT)
                else:
                    nc.vector.tensor_scalar(
                        v_st[cc - 4][:, 512 * st:512 * (st + 1)], cps[:],
                        b, None, ADD)
            return epi

        def xo_epi(cc):
            def epi(st, cps):
                osb = rel_pool.tile([128, 512], F32, tag="osb", name="osb")
                nc.vector.tensor_scalar(
                    osb[:], cps[:], bcol[:, 6 + cc:7 + cc], None, ADD)
                nc.sync.dma_start(
                    out_d[128 * cc:128 * (cc + 1),
                          512 * st:512 * (st + 1)], osb[:])
            return epi

        blocks = {
            "q0": Block(wq_d[0], qkv_epi(0), "q0"),  # ws pre-loaded above
        }
        blocks["q0"].ws = wq0
        blocks.update({
            "k0": Block(wq_d[2], qkv_epi(2), "k0"),
            "v0": Block(wq_d[4], qkv_epi(4), "v0"),
            "q1": Block(wq_d[1], qkv_epi(1), "q1"),
            "k1": Block(wq_d[3], qkv_epi(3), "k1"),
            "v1": Block(wq_d[5], qkv_epi(5), "v1"),
            "xo0": Block(wc_d[0], xo_epi(0), "xo0"),
            "xo1": Block(wc_d[1], xo_epi(1), "xo1"),
        })

        # SWDGE queue order: k0/v0 weights first (needed in pre-phase),
        # then krc, i128, wattT, then the head-phase weights.
        blocks["k0"].prefetch()
        for t in range(2):
            nc.gpsimd.dma_start(krc2[t][:], krc_d[t])
        blocks["v0"].prefetch()
        for i in range(2):
            nc.gpsimd.dma_start(wattT_sb[i][:], wattT_d[i])
        # q1/k1 prefetch now; v1/xo after the pre-phase staging emission so
        # the Pool engine reaches the (data-gated) staging transposes
        # before grinding through more SWDGE triggers
        blocks["q1"].prefetch()
        blocks["k1"].prefetch()

        # ---------- rel tables: 4 heads per matmul pass (row+col tiled,
        # K=32 M=64 tiles at (32g, 64*(g%2))) + skew bounce ----------
        rsb_t = {}
        relp_done = set()
        skew_written = set()

        def rel_pass(qt, tab, st):
            # bank bk holds heads (2bk, 2bk+1) of the half at partition
            # offsets 0/64; evacuated as ONE full 128-row copy per bank
            # (DVE/ACT time scales with free size, not partitions)
            qsrc = q_st[qt] if tab == 0 else q_nat[qt]
            banks = [psum.tile([128, 512], F32, tag="cps",
                               name=f"rp{qt}{tab}{st}{b}") for b in range(2)]
            for g in range(4):
                bk, c = g // 2, g % 2
                nc.tensor.matmul(
                    banks[bk][64 * c:64 * c + 64, :],
                    krc2[tab][32 * g:32 * g + 32, 64 * c:64 * c + 64],
                    qsrc[32 * g:32 * g + 32, 512 * st:512 * (st + 1)],
                    start=True, stop=True,
                    tile_position=(32 * g, 64 * c), skip_group_check=True)
            for bk in range(2):
                key = (qt, bk, tab)
                if key not in rsb_t:
                    rsb_t[key] = rel_pool.tile(
                        [128, HW], BF16, tag="rsb", name=f"rsb{qt}{bk}{tab}",
                        bufs=8)
                rsb = rsb_t[key]
                # qt=0: bank0 on ACT, bank1 on DVE (parallel); qt=1: DVE
                # (ACT is running the exp stream mid-phase)
                if qt == 0 and bk == 0:
                    nc.scalar.activation(
                        rsb[:, 512 * st:512 * (st + 1)],
                        banks[bk][:], mybir.ActivationFunctionType.Copy)
                else:
                    nc.vector.tensor_copy(
                        rsb[:, 512 * st:512 * (st + 1)], banks[bk][:])

        def do_relpass(qt, tab, st):
            if (qt, tab, st) not in relp_done:
                relp_done.add((qt, tab, st))
                rel_pass(qt, tab, st)
                return 2
            return 0

        def skew_write(h, tab):
            # ALL skew triggers live on the SP engine: ACT engine does
            # nothing but exp + rel copies (a 2016-descriptor dma_start
            # costs 1-5us of engine time and head-of-line blocks the FIFO)
            if (h, tab) in skew_written:
                return
            skew_written.add((h, tab))
            rsb = rsb_t[(h // 4, (h % 4) // 2, tab)]
            row0 = 64 * (h % 2)
            src = rsb[row0:row0 + 63, :].rearrange("p (a b) -> p a b", a=32)
            dst = AP(skw[h][tab].tensor, 0,
                     [[HW, 63], [1056, 32], [1, 32]])
            nc.sync.dma_start(dst, src)

        # ---------- v -> vT transpose steps (XBAR DMA transpose frees the
        # PE; it only writes contiguous 2D, so bounce via scratch and DVE
        # into the interleaved [v|ones] layout) ----------
        def vt_step(u, half):
            scr = xfer.tile([128, 128], BF16, tag="vtt",
                            name=f"vt{u}{half}", bufs=2)
            nc.sync.dma_start_transpose(
                scr[:], v_st[half][:, 128 * u:128 * (u + 1)])
            dst = vT_t[u][:].rearrange("p (h j) -> p h j", h=9)
            nc.vector.tensor_copy(
                dst[:, 4 * half:4 * (half + 1), 0:32],
                scr[:].rearrange("p (h d) -> p h d", h=4))

        # ---------- filler queue (with pull-based forcing) ----------
        vt_done = set()

        def do_vt(u, half):
            if (u, half) not in vt_done:
                vt_done.add((u, half))
                vt_step(u, half)
                return 1
            return 0

        fillers = (
            [("blk", "v0")]
            + [("vt", u, 0) for u in range(8)]
            + [("blk", "q1")]
            + [("relp", 1, tab, st) for tab in range(2) for st in range(2)]
            + [("blk", "k1"), ("blk", "v1")]
            + [("vt", u, 1) for u in range(8)]
            + [("blk", "xo0"), ("blk", "xo1")]
        )

        def emit_filler(budget):
            while budget > 0 and fillers:
                item = fillers[0]
                if item[0] == "blk":
                    blk = blocks[item[1]]
                    if blk.done:
                        fillers.pop(0)
                        continue
                    got = blk.emit(budget)
                    budget -= got
                    if blk.done:
                        fillers.pop(0)
                elif item[0] == "relp":
                    budget -= do_relpass(*item[1:])
                    fillers.pop(0)
                else:
                    do_vt(*item[1:])  # XBAR DMA: no PE cost
                    fillers.pop(0)

        def force_block(name):
            blk = blocks[name]
            if not blk.done:
                blk.emit(100)

        def ensure_stage_deps(h):
            if h >= 4:
                force_block("q1")
                force_block("k1")
            for tab in range(2):
                for st in range(2):
                    do_relpass(h // 4, tab, st)
            skew_write(h, 0)
            skew_write(h, 1)

        # ---------- head staging ----------
        ah_t = {}

        def stage_reads(h):
            ensure_stage_deps(h)
            slot = h % 4
            rtt = rt_r[slot]
            skr = AP(skw[h][0].tensor, 31 * HW,
                     [[HW, 32], [32, 32], [1, 32]])
            nc.sync.dma_start(
                rtt[32:64, :].rearrange("p (a b) -> p a b", a=32), skr)
            ah = xfer.tile([32, HW], BF16, tag="ah", name=f"ah{h}", bufs=4)
            ah_t[h] = ah
            skr1 = AP(skw[h][1].tensor, 31 * HW,
                      [[HW, 32], [32, 32], [1, 32]])
            nc.sync.dma_start(
                ah[:].rearrange("p (a b) -> p a b", a=32), skr1)

        def stage_tr(h):
            # pre-phase staging transposes on DVE (~1us); the Pool engine
            # version measures ~3.5us and sits behind SWDGE triggers
            rtt = rt_r[h % 4]
            ah = ah_t.pop(h)
            meng = nc.vector if h < 4 else nc.gpsimd
            meng.tensor_copy(
                rtt[64:96, :].rearrange("p (c r) -> p c r", c=32),
                ah[:].rearrange("p (r c) -> p c r", c=32))

        def stage_skew(h):
            stage_reads(h)
            stage_tr(h)

        def stage_lt(h):
            # emitted after the rel rsb copies so these (k0-gated) copies
            # don't block the DVE FIFO ahead of the skew-write chain
            slot = h % 4
            lt, rtt = lt_r[slot], rt_r[slot]
            qt, j4 = h // 4, h % 4
            nc.vector.tensor_copy(lt[0:32, :],
                                  k_st[qt][32 * j4:32 * j4 + 32, :])
            nc.vector.tensor_copy(rtt[0:32, :],
                                  q_st[qt][32 * j4:32 * j4 + 32, :])

        def stage_head(h):
            stage_skew(h)
            stage_lt(h)

        # ---------- pre-phase: q0 -> rel passes -> staging h0-h3 ----------
        wps = psum.tile([128, 512], F32, tag="cps", name="warmps")
        for i in range(16):
            nc.tensor.matmul(wps[:], warm[:, 0:128], warm[:],
                             start=True, stop=True)
        blocks["q0"].emit(36)
        blocks["k0"].emit(12)
        for tab in range(2):
            for st in range(2):
                do_relpass(0, tab, st)
        for h in range(4):
            stage_reads(h)
        blocks["k0"].emit(100)
        # lt copies on DVE before the (ah-gated) transposes so the first
        # logits LDW isn't stuck behind them in the DVE FIFO
        for h in range(4):
            stage_lt(h)
        for h in range(4):
            stage_tr(h)
        for name in ("v1", "xo0", "xo1"):
            blocks[name].prefetch()
        blocks["v0"].emit(6)
        # slot 2/3 ring constants: queue positions behind the pre-phase
        # skew traffic, needed only at pair 1 (~20us later)
        nc.sync.dma_start(lt_r[2][32:128, :], ltc_d[:])
        nc.sync.dma_start(rt_r[2][96:128, :], rtz_d[:])
        nc.scalar.dma_start(lt_r[3][32:128, :], ltc_d[:])
        nc.scalar.dma_start(rt_r[3][96:128, :], rtz_d[:])

        # ---------- 1x1 conv: split accumulation ----------
        def final_partial(st):
            # W0 @ attn[heads 0-3]: runs right after pair 1 completes
            for ct in range(2):
                ops = psum.tile([128, 512], F32, tag="cps", name=f"fp{ct}{st}")
                nc.tensor.matmul(
                    ops[:], wattT_sb[0][:, 128 * ct:128 * (ct + 1)],
                    attn[0][:, 512 * st:512 * (st + 1)],
                    start=True, stop=True)
                nc.vector.tensor_scalar(
                    opart[2 * ct + st][:], ops[:], bcol[:, 8 + ct:9 + ct],
                    None, ADD)

        def final_tail(st):
            for ct in range(2):
                ops = psum.tile([128, 512], F32, tag="cps", name=f"ft{ct}{st}")
                nc.tensor.matmul(
                    ops[:], wattT_sb[1][:, 128 * ct:128 * (ct + 1)],
                    attn[1][:, 512 * st:512 * (st + 1)],
                    start=True, stop=True)
                osb = rel_pool.tile([128, 512], F32, tag="osb", name="osb")
                nc.vector.tensor_tensor(
                    osb[:], ops[:], opart[2 * ct + st][:], ADD)
                eng = nc.sync if ct == 0 else nc.scalar
                eng.dma_start(
                    out_d[256 + 128 * ct:256 + 128 * (ct + 1),
                          512 * st:512 * (st + 1)],
                    osb[:])

        # ---------- head phase ----------
        es = {}
        aps = {}

        def emit_logits(p, u):
            for j in range(2):
                s = (2 * p + j) % 4
                lt, rtt = lt_r[s], rt_r[s]
                lps = psum.tile([128, HW], F32, tag="lps",
                                name=f"l{p}_{u}_{j}")
                for mh in range(2):
                    nc.tensor.matmul(
                        lps[:, 512 * mh:512 * (mh + 1)],
                        lt[:, 128 * u:128 * (u + 1)],
                        rtt[:, 512 * mh:512 * (mh + 1)],
                        start=True, stop=True)
                e = e_pool.tile([128, HW], BF16, tag="E", name=f"e{p}{u}{j}")
                nc.scalar.activation(e[:], lps[:], Exp)
                es[(p, u, j)] = e

        def pair_end(p):
            # evacuate att + sums via 32x32 block transposes, one m'-half
            # at a time
            h0, h1 = 2 * p, 2 * p + 1
            g = p // 2
            po0, po1 = 32 * (h0 % 4), 32 * (h1 % 4)
            ap0, ap1 = aps.pop(p)
            for mh, apx in ((0, ap0), (1, ap1)):
                c0, c1 = 512 * mh, 512 * (mh + 1)
                nc.vector.transpose(att_un[g][po0:po0 + 32, c0:c1],
                                    apx[0:32, :])
                nc.vector.transpose(smap[g][po0:po0 + 32, c0:c1],
                                    apx[32:64, :])
                nc.vector.transpose(att_un[g][po1:po1 + 32, c0:c1],
                                    apx[64:96, :])
                nc.vector.transpose(smap[g][po1:po1 + 32, c0:c1],
                                    apx[96:128, :])
                if p % 2 == 1:
                    nc.vector.reciprocal_approx_fast(
                        rmap[g][:, c0:c1], smap[g][:, c0:c1])
                    nc.vector.tensor_tensor(
                        attn[g][:, c0:c1], att_un[g][:, c0:c1],
                        rmap[g][:, c0:c1], MULT)
                    if p == 1:
                        final_partial(mh)
                    if p == 3:
                        final_tail(mh)

        def emit_att(p, u):
            h0, h1 = 2 * p, 2 * p + 1
            if u == 0:
                force_block("v0" if h0 < 4 else "v1")
                aps[p] = (
                    psum.tile([128, 512], F32, tag="ap", name=f"ap0_{p}"),
                    psum.tile([128, 512], F32, tag="ap", name=f"ap1_{p}"))
            ap0, ap1 = aps[p]
            do_vt(u, h0 // 4)
            e0, e1 = es.pop((p, u, 0)), es.pop((p, u, 1))
            for mh, apx in ((0, ap0), (1, ap1)):
                nc.tensor.matmul(
                    apx[0:64, :],
                    vT_t[u][:, 64 * h0:64 * h0 + 64],
                    e0[:, 512 * mh:512 * (mh + 1)],
                    start=(u == 0), stop=(u == 7),
                    tile_position=(0, 0), skip_group_check=True)
                nc.tensor.matmul(
                    apx[64:128, :],
                    vT_t[u][:, 64 * h1:64 * h1 + 64],
                    e1[:, 512 * mh:512 * (mh + 1)],
                    start=(u == 0), stop=(u == 7),
                    tile_position=(0, 64), skip_group_check=True)
            if u == 7:
                pair_end(p)

        groups = [(p, u) for p in range(4) for u in range(8)]
        for i in range(len(groups) + LAG):
            if i < len(groups):
                p, u = groups[i]
                emit_logits(p, u)
                if u == 2 and p in (1, 2):
                    stage_head(2 * p + 2)
                    stage_head(2 * p + 3)
            # att (and pair_end) BEFORE filler: the pair-end transposes
            # must not queue behind this iteration's filler epilogues on
            # the DVE FIFO (they gate the aps PSUM recycle -> PE stalls)
            if i >= LAG:
                emit_att(*groups[i - LAG])
            emit_filler(FILL_PER_U)

        # drain any remaining filler work
        emit_filler(10 ** 6)


def _host_inputs(x, w_conv, b_conv, w_qkv, b_qkv, w_att, b_att,
                 key_rel_w, key_rel_h):
    """Build per-core input maps (host-side layout prep only)."""
    import ml_dtypes
    bf16 = ml_dtypes.bfloat16
    x = np.asarray(x, dtype=np.float32)

    def wT(w, nt):
        # (co, ci, 3, 3) -> (cout_tile, ci_half, ci_local, tap, co_local)
        w = np.asarray(w, dtype=np.float32).reshape(nt, 128, 2, 128, 9)
        return np.ascontiguousarray(w.transpose(0, 2, 3, 4, 1)).astype(bf16)

    wqkvT = wT(w_qkv, 6)
    wconvT = wT(w_conv, 2)
    wattT = np.ascontiguousarray(
        np.asarray(w_att, dtype=np.float32)[:, :, 0, 0].T.reshape(
            2, 128, 256)).astype(bf16)
    bias_cols = np.zeros((128, 10), np.float32)
    bias_cols[:, 0:6] = np.asarray(b_qkv, np.float32).reshape(6, 128).T
    bias_cols[:, 6:8] = np.asarray(b_conv, np.float32).reshape(2, 128).T
    bias_cols[:, 8:10] = np.asarray(b_att, np.float32).reshape(2, 128).T
    n = np.arange(HW)
    ltc = np.zeros((96, HW), np.float32)
    ltc[0:32] = np.arange(32)[:, None] == (n % 32)[None, :]   # I_w
    ltc[32:64] = np.arange(32)[:, None] == (n // 32)[None, :]  # I_h
    krcb2 = np.zeros((2, 128, 128), np.float32)
    for t, kr in ((0, key_rel_w), (1, key_rel_h)):
        krT = np.asarray(kr, np.float32).T * SCALE  # (32, 63), q-scale folded
        for g in range(4):
            krcb2[t, 32 * g:32 * (g + 1),
                  64 * (g % 2):64 * (g % 2) + 63] = krT
    shared = {
        "wqkvT": wqkvT, "wconvT": wconvT, "wattT": wattT,
        "bias_cols": bias_cols,
        "krcb2": krcb2.astype(bf16),
        "ltc": ltc.astype(bf16),
        "rtz": np.zeros((32, HW), bf16),
    }
    xp = np.zeros((B, 2, 128, 34, 34), np.float32)
    xp[:, :, :, 1:33, 1:33] = x.reshape(B, 2, 128, 32, 32)
    xp = xp.reshape(B, 2, 128, 34 * 34).astype(bf16)
    return [dict(shared, x=xp[i]) for i in range(B)]


def get_nc():
    if "nc" not in _cached:
        _cached["nc"] = build_bass()
    return _cached["nc"]


def kernel(x, w_conv, b_conv, w_qkv, b_qkv, w_att, b_att,
           key_rel_w, key_rel_h):
    from concourse.bass_utils import run_bass_kernel_spmd
    nc = get_nc()
    in_maps = _host_inputs(x, w_conv, b_conv, w_qkv, b_qkv, w_att, b_att,
                           key_rel_w, key_rel_h)
    res = run_bass_kernel_spmd(nc, in_maps, list(range(N_CORES)))
    out = np.stack([res.results[i]["out"].reshape(COUT, H, W)
                    for i in range(B)])
    return out
